# revision 5
# baseline (speedup 1.0000x reference)
"""Trainium2 Bass kernel for nn_BlockConvolutionLean.

Computation (see reference):
  features = einsum('nse,te->nst', seq_vector, W)        # 1x1 conv
  blocks of BS=8 along S; out = exclusive-cumsum within block + b_eff
  b_eff = bias with bias[0] doubled at position 0.

Key identity: per 8-token block, Out = L @ X @ W^T + b_eff where L is the
strictly-lower-triangular 8x8 ones matrix.  Both factors are matmuls:

  mmA (cumsum fused into transpose):  Z[e, s'] = sum_s X[s, e] * LT[s, s']
      where LT[s, s'] = 1 iff s, s' in same 8-block and s < s'
      (LT is the 128x128 block-diagonal strictly-upper ones matrix).
      lhsT = X chunk [s(128 part), e(128)] -- natural DMA layout!
      rhs  = LT [s(128 part), s'(128)]
      => Z = (L@X)^T lands e-on-partitions, exactly what mmB needs.

  mmB (projection): O[s', t] = sum_e Z[e, s'] * WT[e, t], K=E=256 via two
      accumulating matmuls.  O is token-on-partition => contiguous store.

  bias: per-partition bias add fused into the PSUM->SBUF eviction
      (ScalarE activation Identity with bias AP [128,1], b_eff[p % 8]).

Sharding: data-parallel over N=8 batches, one batch per NeuronCore.
"""

import os

import numpy as np

import concourse.bass as bass
import concourse.mybir as mybir
import concourse.tile as tile
from concourse import bacc
from concourse.bass_utils import run_bass_kernel_spmd

N, S, E, BS = 8, 8192, 256, 8
P = 128                 # tokens per tile / partitions
PACK = 4                # token-tiles per DMA chunk
CHUNK = P * PACK        # tokens per chunk
NCHUNK = S // CHUNK
NCORES = 8

# Precision mode:
#   f32   : everything float32 (exact, PE-bound ~4 cyc/row)
#   f32r  : matmuls in float32r (single-pass fp32, reduced internal precision)
#   bf16  : matmul operands cast to bf16 (fp32 accumulation)
#   mixed : mmA in bf16, mmB in f32r
MODE = os.environ.get("BCONV_MODE", "f32r")

_cache = {}


def _build_nc(mode: str):
    f32 = mybir.dt.float32
    f32r = mybir.dt.float32r
    bf16 = mybir.dt.bfloat16

    # dtype of SBUF tiles feeding mmA (x, lt) and mmB (z, wt)
    if mode == "f32":
        a_dt, b_dt = f32, f32
    elif mode == "f32r":
        a_dt, b_dt = f32, f32     # bitcast to f32r at matmul call sites
    elif mode == "bf16":
        a_dt, b_dt = bf16, bf16
    elif mode == "mixed":
        a_dt, b_dt = bf16, f32    # mmB via f32r bitcast
    else:
        raise ValueError(mode)
    a_mm = f32r if mode == "f32r" else a_dt
    b_mm = f32r if mode in ("f32r", "mixed") else b_dt

    nc = bacc.Bacc(
        "TRN2", target_bir_lowering=False, debug=False, num_devices=NCORES)
    x = nc.dram_tensor("x", [S, E], f32, kind="ExternalInput")
    lt = nc.dram_tensor("lt", [P, P], a_dt, kind="ExternalInput")
    wt = nc.dram_tensor("wt", [P, 2 * E], b_dt, kind="ExternalInput")
    beff = nc.dram_tensor("beff", [P, 1], f32, kind="ExternalInput")
    y = nc.dram_tensor("y", [S, E], f32, kind="ExternalOutput")

    x_cast = a_dt != f32
    ident = mybir.ActivationFunctionType.Identity

    with tile.TileContext(nc) as tc:
        with (
            tc.tile_pool(name="const", bufs=1) as constp,
            tc.tile_pool(name="xin", bufs=3) as xin,
            tc.tile_pool(name="zsb", bufs=4) as zsbp,
            tc.tile_pool(name="yout", bufs=3) as yout,
            tc.tile_pool(name="zps", bufs=4, space="PSUM") as zps,
            tc.tile_pool(name="ops", bufs=3, space="PSUM") as ops,
        ):
            lt_sb = constp.tile([P, P], a_dt)
            nc.sync.dma_start(lt_sb[:], lt[:, :])
            wt_sb = constp.tile([P, 2 * E], b_dt)
            nc.sync.dma_start(wt_sb[:], wt[:, :])
            beff_sb = constp.tile([P, 1], f32)
            nc.sync.dma_start(beff_sb[:], beff[:, :])

            for c in range(NCHUNK):
                xsl = x[c * CHUNK:(c + 1) * CHUNK, :].rearrange(
                    "(i p) e -> p i e", p=P)
                xt = xin.tile([P, PACK * E], a_dt)
                if x_cast:
                    nc.gpsimd.dma_start(xt[:], xsl)
                else:
                    nc.sync.dma_start(xt[:], xsl)

                ot = yout.tile([P, PACK * E], f32)
                for j in range(PACK):
                    zt = zsbp.tile([P, 2 * P], b_dt)
                    for k in range(2):
                        zp = zps.tile([P, P], f32)
                        nc.tensor.matmul(
                            zp[:],
                            xt[:, j * E + k * P: j * E + (k + 1) * P].bitcast(a_mm),
                            lt_sb[:].bitcast(a_mm),
                            start=True, stop=True,
                        )
                        nc.vector.tensor_copy(zt[:, k * P:(k + 1) * P], zp[:])
                    op = ops.tile([P, E], f32)
                    nc.tensor.matmul(
                        op[:], zt[:, 0:P].bitcast(b_mm),
                        wt_sb[:, 0:E].bitcast(b_mm),
                        start=True, stop=False)
                    nc.tensor.matmul(
                        op[:], zt[:, P:2 * P].bitcast(b_mm),
                        wt_sb[:, E:2 * E].bitcast(b_mm),
                        start=False, stop=True)
                    nc.scalar.activation(
                        ot[:, j * E:(j + 1) * E], op[:], ident, bias=beff_sb[:])

                ysl = y[c * CHUNK:(c + 1) * CHUNK, :].rearrange(
                    "(i p) e -> p i e", p=P)
                nc.sync.dma_start(ysl, ot[:])
    nc.compile()
    return nc


def _np_dt(dt):
    import ml_dtypes
    return {"bfloat16": ml_dtypes.bfloat16}.get(dt, np.float32)


def _host_consts(W: np.ndarray, b: np.ndarray, mode: str):
    if mode == "f32" or mode == "f32r":
        a_np, b_np = np.float32, np.float32
    elif mode == "bf16":
        a_np, b_np = _np_dt("bfloat16"), _np_dt("bfloat16")
    elif mode == "mixed":
        a_np, b_np = _np_dt("bfloat16"), np.float32
    else:
        raise ValueError(mode)

    idx = np.arange(P)
    blk = idx // BS
    LT = ((blk[:, None] == blk[None, :]) & (idx[:, None] < idx[None, :]))
    LT = np.ascontiguousarray(LT.astype(a_np))

    WT = W.T.astype(np.float32)                      # [E, T]
    WTP = np.concatenate([WT[0:P, :], WT[P:2 * P, :]], axis=1)  # [128, 512]
    WTP = np.ascontiguousarray(WTP.astype(b_np))

    be = b.astype(np.float64).copy()
    be[0] += be[0]
    BEFF = np.ascontiguousarray(
        be[idx % BS].astype(np.float32).reshape(P, 1))
    return LT, WTP, BEFF


def kernel(seq_vector, W, bias):
    mode = MODE
    x = np.ascontiguousarray(np.asarray(seq_vector, dtype=np.float32))
    W = np.asarray(W, dtype=np.float32)
    b = np.asarray(bias, dtype=np.float32)
    assert x.shape == (N, S, E)

    LT, WTP, BEFF = _host_consts(W, b, mode)

    if mode not in _cache:
        _cache[mode] = _build_nc(mode)
    nc = _cache[mode]

    in_maps = [
        {"x": x[i], "lt": LT, "wt": WTP, "beff": BEFF}
        for i in range(NCORES)
    ]
    res = run_bass_kernel_spmd(nc, in_maps, core_ids=list(range(NCORES)))
    out = np.stack([r["y"] for r in res.results], axis=0)
    return out.reshape(N, S, E)


# revision 7
# speedup vs baseline: 1.5693x; 1.5693x over previous
"""Trainium2 Bass kernel for nn_BlockConvolutionLean.

Computation (see reference):
  features = einsum('nse,te->nst', seq_vector, W)        # 1x1 conv
  blocks of BS=8 along S; out = exclusive-cumsum within block + b_eff
  b_eff = bias with bias[0] doubled at position 0.

Key identity: per 8-token block, Out = L @ X @ W^T + b_eff where L is the
strictly-lower-triangular 8x8 ones matrix.  Both factors are matmuls:

  mmA (cumsum fused into transpose):  Z[e, s'] = sum_s X[s, e] * LT[s, s']
      where LT[s, s'] = 1 iff s, s' in same 8-block and s < s'
      (LT is the 128x128 block-diagonal strictly-upper ones matrix).
      lhsT = X chunk [s(128 part), e(128)] -- natural DMA layout!
      rhs  = LT [s(128 part), s'(128)]
      => Z = (L@X)^T lands e-on-partitions, exactly what mmB needs.

  mmB (projection): O[s', t] = sum_e Z[e, s'] * WT[e, t], K=E=256 via two
      accumulating matmuls.  O is token-on-partition => contiguous store.

  bias: per-partition bias add fused into the PSUM->SBUF eviction
      (ScalarE activation Identity with bias AP [128,1], b_eff[p % 8]).

Sharding: data-parallel over N=8 batches, one batch per NeuronCore.
"""

import os

import numpy as np

import concourse.bass as bass
import concourse.mybir as mybir
import concourse.tile as tile
from concourse import bacc
from concourse.bass_utils import run_bass_kernel_spmd

N, S, E, BS = 8, 8192, 256, 8
P = 128                 # tokens per tile / partitions
PACK = 4                # token-tiles per DMA chunk
CHUNK = P * PACK        # tokens per chunk
NCHUNK = S // CHUNK
NCORES = 8

# Precision mode:
#   f32   : everything float32 (exact, PE-bound ~4 cyc/row)
#   f32r  : matmuls in float32r (single-pass fp32, reduced internal precision)
#   bf16  : matmul operands cast to bf16 (fp32 accumulation)
#   mixed : mmA in bf16, mmB in f32r
MODE = os.environ.get("BCONV_MODE", "f32r")

_cache = {}


def _build_nc(mode: str):
    f32 = mybir.dt.float32
    f32r = mybir.dt.float32r
    bf16 = mybir.dt.bfloat16

    # dtype of SBUF tiles feeding mmA (x, lt) and mmB (z, wt)
    if mode == "f32":
        a_dt, b_dt = f32, f32
    elif mode == "f32r":
        a_dt, b_dt = f32r, f32r
    elif mode == "bf16":
        a_dt, b_dt = bf16, bf16
    elif mode == "mixed":
        a_dt, b_dt = bf16, f32r
    else:
        raise ValueError(mode)

    nc = bacc.Bacc(
        "TRN2", target_bir_lowering=False, debug=False, num_devices=NCORES)
    # x dram dtype matches the tile dtype for f32r (bit-identical, no cast);
    # stays f32 for bf16 (SWDGE casts in flight).
    x_dram_dt = a_dt if a_dt == f32r else f32
    x = nc.dram_tensor("x", [S, E], x_dram_dt, kind="ExternalInput")
    lt = nc.dram_tensor("lt", [P, P], a_dt, kind="ExternalInput")
    wt = nc.dram_tensor("wt", [P, 2 * E], b_dt, kind="ExternalInput")
    beff = nc.dram_tensor("beff", [P, 1], f32, kind="ExternalInput")
    y = nc.dram_tensor("y", [S, E], f32, kind="ExternalOutput")

    x_cast = a_dt == bf16
    ident = mybir.ActivationFunctionType.Identity

    with tile.TileContext(nc) as tc:
        with (
            tc.tile_pool(name="const", bufs=1) as constp,
            tc.tile_pool(name="xin", bufs=3) as xin,
            tc.tile_pool(name="zsb", bufs=4) as zsbp,
            tc.tile_pool(name="yout", bufs=3) as yout,
            tc.tile_pool(name="zps", bufs=4, space="PSUM") as zps,
            tc.tile_pool(name="ops", bufs=3, space="PSUM") as ops,
        ):
            lt_sb = constp.tile([P, P], a_dt)
            nc.sync.dma_start(lt_sb[:], lt[:, :])
            wt_sb = constp.tile([P, 2 * E], b_dt)
            nc.sync.dma_start(wt_sb[:], wt[:, :])
            beff_sb = constp.tile([P, 1], f32)
            nc.sync.dma_start(beff_sb[:], beff[:, :])

            for c in range(NCHUNK):
                xsl = x[c * CHUNK:(c + 1) * CHUNK, :].rearrange(
                    "(i p) e -> p i e", p=P)
                xt = xin.tile([P, PACK * E], a_dt)
                if x_cast:
                    nc.gpsimd.dma_start(xt[:], xsl)
                else:
                    nc.sync.dma_start(xt[:], xsl)

                ot = yout.tile([P, PACK * E], f32)
                for j in range(PACK):
                    zt = zsbp.tile([P, 2 * P], b_dt)
                    for k in range(2):
                        zp = zps.tile([P, P], f32)
                        nc.tensor.matmul(
                            zp[:],
                            xt[:, j * E + k * P: j * E + (k + 1) * P],
                            lt_sb[:],
                            start=True, stop=True,
                        )
                        nc.vector.tensor_copy(zt[:, k * P:(k + 1) * P], zp[:])
                    op = ops.tile([P, E], f32)
                    nc.tensor.matmul(
                        op[:], zt[:, 0:P],
                        wt_sb[:, 0:E],
                        start=True, stop=False)
                    nc.tensor.matmul(
                        op[:], zt[:, P:2 * P],
                        wt_sb[:, E:2 * E],
                        start=False, stop=True)
                    nc.scalar.activation(
                        ot[:, j * E:(j + 1) * E], op[:], ident, bias=beff_sb[:])

                ysl = y[c * CHUNK:(c + 1) * CHUNK, :].rearrange(
                    "(i p) e -> p i e", p=P)
                nc.sync.dma_start(ysl, ot[:])
    nc.compile()
    return nc


def _np_dt(dt):
    import ml_dtypes
    return {"bfloat16": ml_dtypes.bfloat16}.get(dt, np.float32)


def _host_consts(W: np.ndarray, b: np.ndarray, mode: str):
    if mode == "f32" or mode == "f32r":
        a_np, b_np = np.float32, np.float32
    elif mode == "bf16":
        a_np, b_np = _np_dt("bfloat16"), _np_dt("bfloat16")
    elif mode == "mixed":
        a_np, b_np = _np_dt("bfloat16"), np.float32
    else:
        raise ValueError(mode)

    idx = np.arange(P)
    blk = idx // BS
    LT = ((blk[:, None] == blk[None, :]) & (idx[:, None] < idx[None, :]))
    LT = np.ascontiguousarray(LT.astype(a_np))

    WT = W.T.astype(np.float32)                      # [E, T]
    WTP = np.concatenate([WT[0:P, :], WT[P:2 * P, :]], axis=1)  # [128, 512]
    WTP = np.ascontiguousarray(WTP.astype(b_np))

    be = b.astype(np.float64).copy()
    be[0] += be[0]
    BEFF = np.ascontiguousarray(
        be[idx % BS].astype(np.float32).reshape(P, 1))
    return LT, WTP, BEFF


def kernel(seq_vector, W, bias):
    mode = MODE
    x = np.ascontiguousarray(np.asarray(seq_vector, dtype=np.float32))
    W = np.asarray(W, dtype=np.float32)
    b = np.asarray(bias, dtype=np.float32)
    assert x.shape == (N, S, E)

    LT, WTP, BEFF = _host_consts(W, b, mode)

    if mode not in _cache:
        _cache[mode] = _build_nc(mode)
    nc = _cache[mode]

    in_maps = [
        {"x": x[i], "lt": LT, "wt": WTP, "beff": BEFF}
        for i in range(NCORES)
    ]
    res = run_bass_kernel_spmd(nc, in_maps, core_ids=list(range(NCORES)))
    out = np.stack([r["y"] for r in res.results], axis=0)
    return out.reshape(N, S, E)
